# revision 3
# baseline (speedup 1.0000x reference)
"""Trainium2 Bass kernel for a dense (length-1 sequence) Mamba block.

The reference computation reduces algebraically to:
    z   = x @ in_w                                  # (B, d_inner)
    g   = silu(z * c + b_eff)                       # per-channel scale/bias
    out = g @ out_w + out_b                         # (B, d_model)
with
    c     = conv_w[:, -1] + softplus(dt) * sum(B*C, -1) + Dp
    b_eff = (in_b * c) + conv_b
(c, b_eff are tiny per-channel vectors, computed on host.)

Strategy: data-parallel over 8 NeuronCores (batch 32768 -> 8 x 4096).
All matmul operands are bf16 (tolerance is rel 2e-2; bf16 end-to-end
measures ~3e-3).  x is transposed and cast to bf16 on the host, so the
kernel streams x^T tiles straight from DRAM -- no PE transposes.
Per core, batch is processed in tiles of BT rows:
  phase M1: z^T[di, b] accumulated over d_model with in_w tiles as the
            stationary operand; Silu fused on ScalarE with per-partition
            scale/bias -> g [di, b] (bf16)
  phase M2: out[b, dm] accumulated over d_inner with g slices as the
            stationary operand and out_w tiles moving (natural output
            layout); out_b added on drain (VectorE), fp32 out store.
in_w is host-packed to [128, NDI, KT*128] bf16 so each stationary tile
loads as one contiguous 4KB-per-partition DMA line.
"""

import numpy as np

import concourse.bass as bass
import concourse.tile as tile
from concourse import bacc, mybir
from concourse.bass_utils import run_bass_kernel_spmd

P = 128
B_FULL = 32768
DM = 2048
DI = 4096
N_CORES = 8
BS = B_FULL // N_CORES  # rows per core

F32 = mybir.dt.float32
BF16 = mybir.dt.bfloat16
SILU = mybir.ActivationFunctionType.Silu


def build_nc(cfg):
    """Build the per-core Bass module. cfg: dict(BT=...)"""
    BT = cfg["BT"]

    NBT = BS // BT          # batch tiles per core
    NB_SUB = BT // P        # 128-row subtiles per batch tile
    KT = DM // P            # k-tiles for matmul 1
    NDI = DI // P           # d_inner chunks of 128
    NDM = DM // 512         # d_model chunks of 512
    H = BT // 512           # moving-dim chunks for matmul 1
    GRP = 4                 # psum banks used by M2 accumulation
    NGRP = NB_SUB // GRP
    DIG = 4                 # d_inner chunks per out_w DMA batch

    nc = bacc.Bacc("TRN2", target_bir_lowering=False, debug=False,
                   num_devices=N_CORES)

    xt_d = nc.dram_tensor("xt", [DM, BS], BF16, kind="ExternalInput").ap()
    iw_d = nc.dram_tensor("iw", [P, NDI * KT * P], BF16,
                          kind="ExternalInput").ap()
    ow_d = nc.dram_tensor("ow", [DI, DM], BF16, kind="ExternalInput").ap()
    c_d = nc.dram_tensor("cpb", [P, NDI], F32, kind="ExternalInput").ap()
    b_d = nc.dram_tensor("bpb", [P, NDI], F32, kind="ExternalInput").ap()
    ob_d = nc.dram_tensor("ob", [P, DM], F32, kind="ExternalInput").ap()
    out_d = nc.dram_tensor("out", [BS, DM], F32, kind="ExternalOutput").ap()

    with tile.TileContext(nc) as tc:
        with (
            tc.tile_pool(name="const", bufs=1) as const,
            tc.tile_pool(name="xT", bufs=2) as xTp,
            tc.tile_pool(name="g", bufs=1) as gp,
            tc.tile_pool(name="iw", bufs=3) as iwp,
            tc.tile_pool(name="ow", bufs=3) as owp,
            tc.tile_pool(name="osb", bufs=2) as osbp,
            tc.tile_pool(name="psZ", bufs=3, space="PSUM") as psZ,
            tc.tile_pool(name="psO", bufs=5, space="PSUM") as psO,
        ):
            c_sb = const.tile([P, NDI], F32)
            nc.sync.dma_start(c_sb[:], c_d)
            b_sb = const.tile([P, NDI], F32)
            nc.sync.dma_start(b_sb[:], b_d)
            # ob_sb is only needed at the first M2 drain (~250us in); load
            # it lazily so the 1MB transfer doesn't delay the first x^T/iw
            # tiles during the prologue.
            ob_sb = const.tile([P, DM], F32)
            ob_loaded = [False]

            g = gp.tile([P, NDI, BT], BF16)

            NKQ = 4                 # x^T tile DMA chunks

            def load_xT_chunk(xT, t, kq):
                nc.gpsimd.dma_start(
                    xT[:, kq * (KT // NKQ):(kq + 1) * (KT // NKQ), :],
                    xt_d[kq * (KT // NKQ) * P:(kq + 1) * (KT // NKQ) * P,
                         t * BT:(t + 1) * BT].rearrange(
                             "(kt p) b -> p kt b", p=P))

            xT_cur = xTp.tile([P, KT, BT], BF16, tag="xT", name="xT")
            for kq in range(NKQ):
                load_xT_chunk(xT_cur, 0, kq)
            for t in range(NBT):
                # ---- phase M1: z^T = in_w^T @ x^T ; g = silu(z*c + b) ----
                # h-outer so the second psum tile isn't needed until the
                # first 16-matmul chain is done (hides activation drain).
                for di in range(NDI):
                    iw_t = iwp.tile([P, KT, P], BF16)
                    nc.scalar.dma_start(
                        iw_t[:],
                        iw_d[:, di * KT * P:(di + 1) * KT * P].rearrange(
                            "p (kt m) -> p kt m", m=P))
                    for h in range(H):
                        zp = psZ.tile([P, 512], F32, tag="zp", name="zp")
                        for kt in range(KT):
                            nc.tensor.matmul(
                                zp[:],
                                iw_t[:, kt, :],
                                xT_cur[:, kt, h * 512:(h + 1) * 512],
                                start=(kt == 0), stop=(kt == KT - 1))
                        nc.scalar.activation(
                            g[:, di, h * 512:(h + 1) * 512], zp[:], SILU,
                            bias=b_sb[:, di:di + 1], scale=c_sb[:, di:di + 1])

                if not ob_loaded[0]:
                    nc.sync.dma_start(ob_sb[:], ob_d)
                    ob_loaded[0] = True

                # next batch tile's x^T tile: chunks spread across M2's dmc
                # iterations so the 4MB prefetch never bursts ahead of the
                # ow_t stream on the DMA rings.
                if t + 1 < NBT:
                    xT_next = xTp.tile([P, KT, BT], BF16, tag="xT",
                                       name="xT")

                # ---- phase M2: out = g^T @ out_w + out_b ----
                for dmc in range(NDM):
                    if t + 1 < NBT:
                        load_xT_chunk(xT_next, t + 1, dmc)
                    for grp in range(NGRP):
                        ops = [psO.tile([P, 512], F32, tag="ps_o",
                                        name=f"ops_{j}")
                               for j in range(GRP)]
                        for dg in range(NDI // DIG):
                            ow_t = owp.tile([P, DIG, 512], BF16)
                            nc.sync.dma_start(
                                ow_t[:],
                                ow_d[dg * DIG * P:(dg + 1) * DIG * P,
                                     dmc * 512:(dmc + 1) * 512].rearrange(
                                         "(s p) n -> p s n", p=P))
                            for s in range(DIG):
                                di = dg * DIG + s
                                for j in range(GRP):
                                    bs = grp * GRP + j
                                    nc.tensor.matmul(
                                        ops[j][:],
                                        g[:, di, bs * P:(bs + 1) * P],
                                        ow_t[:, s, :],
                                        start=(di == 0),
                                        stop=(di == NDI - 1))
                        last = (t == NBT - 1 and dmc == NDM - 1
                                and grp == NGRP - 1)
                        r0 = t * BT + grp * GRP * P
                        if last:
                            # split the final drain per-j so the last
                            # stores overlap the tail matmuls
                            for j in range(GRP):
                                osb = osbp.tile([P, 512], F32, tag="osb1",
                                                name="osb1")
                                nc.vector.tensor_tensor(
                                    osb[:], ops[j][:],
                                    ob_sb[:, dmc * 512:(dmc + 1) * 512],
                                    mybir.AluOpType.add)
                                nc.scalar.dma_start(
                                    out_d[r0 + j * P:r0 + (j + 1) * P,
                                          dmc * 512:(dmc + 1) * 512],
                                    osb[:])
                        else:
                            osb = osbp.tile([P, GRP, 512], F32)
                            for j in range(GRP):
                                nc.vector.tensor_tensor(
                                    osb[:, j, :], ops[j][:],
                                    ob_sb[:, dmc * 512:(dmc + 1) * 512],
                                    mybir.AluOpType.add)
                            nc.scalar.dma_start(
                                out_d[r0:r0 + GRP * P,
                                      dmc * 512:(dmc + 1) * 512].rearrange(
                                          "(s p) n -> p s n", p=P),
                                osb[:])
                if t + 1 < NBT:
                    xT_cur = xT_next
    nc.compile()
    return nc


_NC_CACHE = {}


def _get_nc(key):
    if key not in _NC_CACHE:
        cfg = dict(BT=key[0])
        _NC_CACHE[key] = build_nc(cfg)
    return _NC_CACHE[key]


CONFIG = (1024,)


def _softplus(v):
    return np.logaddexp(0.0, v)


def prep_inputs(x, in_w, in_b, conv_w, conv_b, A_log, B, C, Dp, dt,
                out_w, out_b):
    """Host-side preprocessing: per-core input dicts (shared weight arrays)."""
    import ml_dtypes
    BF = ml_dtypes.bfloat16
    x = np.asarray(x, dtype=np.float32)
    in_w = np.asarray(in_w, dtype=np.float32)
    out_w = np.asarray(out_w, dtype=np.float32)

    # host precompute of the per-channel SSM/conv collapse
    c = (np.asarray(conv_w, np.float32)[:, -1]
         + _softplus(np.asarray(dt, np.float32))
         * np.sum(np.asarray(B, np.float32) * np.asarray(C, np.float32), -1)
         + np.asarray(Dp, np.float32))
    b_eff = np.asarray(in_b, np.float32) * c + np.asarray(conv_b, np.float32)

    # [128, DI//128] partition-major layouts for per-partition scale/bias
    c_pb = np.ascontiguousarray(c.reshape(DI // P, P).T)
    b_pb = np.ascontiguousarray(b_eff.reshape(DI // P, P).T)
    ob_rep = np.ascontiguousarray(
        np.broadcast_to(np.asarray(out_b, np.float32), (P, DM)))

    # x^T per core in bf16: [8, DM, BS]
    xt = np.ascontiguousarray(
        x.reshape(N_CORES, BS, DM).transpose(0, 2, 1).astype(BF))
    # in_w packed [p, di, kt*128] so each stationary tile is one
    # contiguous 4KB-per-partition line: in_w[kt*128+p, di*128+m]
    KT = DM // P
    NDI = DI // P
    iw_pack = np.ascontiguousarray(
        in_w.reshape(KT, P, NDI, P).transpose(1, 2, 0, 3)
        .reshape(P, NDI * KT * P).astype(BF))
    ow_arr = np.ascontiguousarray(out_w.astype(BF))

    in_maps = []
    for i in range(N_CORES):
        in_maps.append({
            "xt": xt[i],
            "iw": iw_pack,
            "ow": ow_arr,
            "cpb": c_pb,
            "bpb": b_pb,
            "ob": ob_rep,
        })
    return in_maps


def kernel(x, in_w, in_b, conv_w, conv_b, A_log, B, C, Dp, dt, out_w, out_b):
    in_maps = prep_inputs(x, in_w, in_b, conv_w, conv_b, A_log, B, C, Dp,
                          dt, out_w, out_b)
    nc = _get_nc(CONFIG)
    out = np.empty((B_FULL, DM), dtype=np.float32)
    try:
        res = run_bass_kernel_spmd(nc, in_maps, core_ids=list(range(N_CORES)))
        for i in range(N_CORES):
            out[i * BS:(i + 1) * BS] = res.results[i]["out"]
    except Exception:
        # The accelerator occasionally hits a transient unrecoverable fault
        # that poisons this process's PJRT client; a fresh process recovers.
        # Retry the device execution in a subprocess.
        _run_in_subprocess(in_maps, out)
    return out


def _run_in_subprocess(in_maps, out):
    import pickle
    import subprocess
    import sys
    import tempfile

    with tempfile.TemporaryDirectory() as td:
        in_path = f"{td}/in.pkl"
        out_path = f"{td}/out.npy"
        with open(in_path, "wb") as f:
            pickle.dump({"config": CONFIG, "in_maps": in_maps}, f,
                        protocol=pickle.HIGHEST_PROTOCOL)
        for attempt in range(3):
            r = subprocess.run(
                [sys.executable, __file__, "--worker", in_path, out_path],
                capture_output=True)
            if r.returncode == 0:
                break
            if attempt == 2:
                raise RuntimeError(
                    f"device worker failed 3x: {r.stderr[-2000:]!r}")
        out[:] = np.load(out_path)


def _worker_main(in_path, out_path):
    import pickle
    with open(in_path, "rb") as f:
        job = pickle.load(f)
    nc = _get_nc(tuple(job["config"]))
    res = run_bass_kernel_spmd(nc, job["in_maps"],
                               core_ids=list(range(N_CORES)))
    out = np.empty((B_FULL, DM), dtype=np.float32)
    for i in range(N_CORES):
        out[i * BS:(i + 1) * BS] = res.results[i]["out"]
    np.save(out_path, out)


if __name__ == "__main__":
    import sys as _sys
    if len(_sys.argv) == 4 and _sys.argv[1] == "--worker":
        _worker_main(_sys.argv[2], _sys.argv[3])


# revision 8
# speedup vs baseline: 1.1423x; 1.1423x over previous
"""Trainium2 Bass kernel for a dense (length-1 sequence) Mamba block.

The reference computation reduces algebraically to:
    z   = x @ in_w                                  # (B, d_inner)
    g   = silu(z * c + b_eff)                       # per-channel scale/bias
    out = g @ out_w + out_b                         # (B, d_model)
with
    c     = conv_w[:, -1] + softplus(dt) * sum(B*C, -1) + Dp
    b_eff = (in_b * c) + conv_b
(c, b_eff are tiny per-channel vectors, computed on host.)

Strategy: data-parallel over 8 NeuronCores (batch 32768 -> 8 x 4096).
All matmul operands are bf16 (tolerance is rel 2e-2; bf16 end-to-end
measures ~3e-3).  x is transposed and cast to bf16 on the host, so the
kernel streams x^T tiles straight from DRAM -- no PE transposes.
Per core, batch is processed in tiles of BT rows:
  phase M1: z^T[di, b] accumulated over d_model with in_w tiles as the
            stationary operand; Silu fused on ScalarE with per-partition
            scale/bias -> g [di, b] (bf16)
  phase M2: out[b, dm] accumulated over d_inner with g slices as the
            stationary operand and out_w tiles moving (natural output
            layout); out_b added on drain (VectorE), fp32 out store.
in_w is host-packed to [128, NDI, KT*128] bf16 so each stationary tile
loads as one contiguous 4KB-per-partition DMA line.
"""

import numpy as np

import concourse.bass as bass
import concourse.tile as tile
from concourse import bacc, mybir
from concourse.bass_utils import run_bass_kernel_spmd

P = 128
B_FULL = 32768
DM = 2048
DI = 4096
N_CORES = 8
BS = B_FULL // N_CORES  # rows per core

F32 = mybir.dt.float32
BF16 = mybir.dt.bfloat16
SILU = mybir.ActivationFunctionType.Silu


def build_nc(cfg):
    """Build the per-core Bass module. cfg: dict(BT=...)"""
    BT = cfg["BT"]

    NBT = BS // BT          # batch tiles per core
    NB_SUB = BT // P        # 128-row subtiles per batch tile
    KT = DM // P            # k-tiles for matmul 1
    NDI = DI // P           # d_inner chunks of 128
    NDM = DM // 512         # d_model chunks of 512
    H = BT // 512           # moving-dim chunks for matmul 1
    GRP = 4                 # psum banks used by M2 accumulation
    NGRP = NB_SUB // GRP
    DIG = 4                 # d_inner chunks per out_w DMA batch

    nc = bacc.Bacc("TRN2", target_bir_lowering=False, debug=False,
                   num_devices=N_CORES)

    xt_d = nc.dram_tensor("xt", [DM, BS], BF16, kind="ExternalInput").ap()
    scr_d = nc.dram_tensor("scr", [1, 512], BF16, kind="Internal").ap()
    iw_d = nc.dram_tensor("iw", [P, NDI * KT * P], BF16,
                          kind="ExternalInput").ap()
    ow_d = nc.dram_tensor("ow", [DI, DM], BF16, kind="ExternalInput").ap()
    c_d = nc.dram_tensor("cpb", [P, NDI], F32, kind="ExternalInput").ap()
    b_d = nc.dram_tensor("bpb", [P, NDI], F32, kind="ExternalInput").ap()
    ob_d = nc.dram_tensor("ob", [P, DM], F32, kind="ExternalInput").ap()
    out_d = nc.dram_tensor("out", [BS, DM], F32, kind="ExternalOutput").ap()

    with tile.TileContext(nc) as tc:
        with (
            tc.tile_pool(name="const", bufs=1) as const,
            tc.tile_pool(name="xT", bufs=2) as xTp,
            tc.tile_pool(name="g", bufs=1) as gp,
            tc.tile_pool(name="iw", bufs=3) as iwp,
            tc.tile_pool(name="ow", bufs=5) as owp,
            tc.tile_pool(name="osb", bufs=2) as osbp,
            tc.tile_pool(name="psZ", bufs=3, space="PSUM") as psZ,
            tc.tile_pool(name="psO", bufs=5, space="PSUM") as psO,
        ):
            c_sb = const.tile([P, NDI], F32)
            nc.sync.dma_start(c_sb[:], c_d)
            b_sb = const.tile([P, NDI], F32)
            nc.sync.dma_start(b_sb[:], b_d)
            # ob_sb is only needed at the first M2 drain (~250us in); load
            # it lazily so the 1MB transfer doesn't delay the first x^T/iw
            # tiles during the prologue.
            ob_sb = const.tile([P, DM], F32)
            ob_loaded = [False]

            g = gp.tile([P, NDI, BT], BF16)

            NKQ = 8                 # x^T tile DMA chunks (kt-axis)

            def load_xT_chunk(xT, t, kq):
                kc = KT // NKQ
                nc.gpsimd.dma_start(
                    xT[:, kq * kc:(kq + 1) * kc, :],
                    xt_d[kq * kc * P:(kq + 1) * kc * P,
                         t * BT:(t + 1) * BT].rearrange(
                             "(kt p) b -> p kt b", p=P))

            # t=0 prologue: load the h=0 half (cols 0:512, all kt) as one
            # DMA so the first M1 chain's working set isn't round-robined
            # with the rest; a tiny fence DMA on the same queue delays the
            # h=1 half until the h=0 half has landed.
            xT_cur = xTp.tile([P, KT, BT], BF16, tag="xT", name="xT")
            nc.gpsimd.dma_start(
                xT_cur[:, :, 0:512],
                xt_d[:, 0:512].rearrange("(kt p) b -> p kt b", p=P))
            nc.gpsimd.dma_start(scr_d[0:1, :], xT_cur[0:1, 0, 0:512])
            nc.gpsimd.dma_start(
                xT_cur[:, :, 512:BT],
                xt_d[:, 512:BT].rearrange("(kt p) b -> p kt b", p=P))
            for t in range(NBT):
                # ---- phase M1: z^T = in_w^T @ x^T ; g = silu(z*c + b) ----
                # h-outer so the second psum tile isn't needed until the
                # first 16-matmul chain is done (hides activation drain).
                for di in range(NDI):
                    if t == 0 and di == NDI // 2 and not ob_loaded[0]:
                        # mid-M1: far from the prologue and the M1->M2
                        # transition, where DMA bandwidth is contended
                        nc.sync.dma_start(ob_sb[:], ob_d)
                        ob_loaded[0] = True
                    iw_t = iwp.tile([P, KT, P], BF16)
                    nc.scalar.dma_start(
                        iw_t[:],
                        iw_d[:, di * KT * P:(di + 1) * KT * P].rearrange(
                            "p (kt m) -> p kt m", m=P))
                    for h in range(H):
                        zp = psZ.tile([P, 512], F32, tag="zp", name="zp")
                        for kt in range(KT):
                            nc.tensor.matmul(
                                zp[:],
                                iw_t[:, kt, :],
                                xT_cur[:, kt, h * 512:(h + 1) * 512],
                                start=(kt == 0), stop=(kt == KT - 1))
                        nc.scalar.activation(
                            g[:, di, h * 512:(h + 1) * 512], zp[:], SILU,
                            bias=b_sb[:, di:di + 1], scale=c_sb[:, di:di + 1])

                # next batch tile's x^T tile: 512KB chunks spread across
                # M2's (dmc, grp) iterations so the 4MB prefetch never
                # bursts ahead of the ow_t stream on the DMA rings.
                if t + 1 < NBT:
                    xT_next = xTp.tile([P, KT, BT], BF16, tag="xT",
                                       name="xT")

                # ---- phase M2: out = g^T @ out_w + out_b ----
                for dmc in range(NDM):
                    for grp in range(NGRP):
                        if t + 1 < NBT:
                            load_xT_chunk(xT_next, t + 1,
                                          dmc * NGRP + grp)
                        ops = [psO.tile([P, 512], F32, tag="ps_o",
                                        name=f"ops_{j}")
                               for j in range(GRP)]
                        for dg in range(NDI // DIG):
                            ow_t = owp.tile([P, DIG, 512], BF16)
                            nc.sync.dma_start(
                                ow_t[:],
                                ow_d[dg * DIG * P:(dg + 1) * DIG * P,
                                     dmc * 512:(dmc + 1) * 512].rearrange(
                                         "(s p) n -> p s n", p=P))
                            for s in range(DIG):
                                di = dg * DIG + s
                                for j in range(GRP):
                                    bs = grp * GRP + j
                                    nc.tensor.matmul(
                                        ops[j][:],
                                        g[:, di, bs * P:(bs + 1) * P],
                                        ow_t[:, s, :],
                                        start=(di == 0),
                                        stop=(di == NDI - 1))
                        last = (t == NBT - 1 and dmc == NDM - 1
                                and grp == NGRP - 1)
                        r0 = t * BT + grp * GRP * P
                        if last:
                            # split the final drain per-j so the last
                            # stores overlap the tail matmuls
                            for j in range(GRP):
                                osb = osbp.tile([P, 512], F32, tag="osb1",
                                                name="osb1")
                                nc.vector.tensor_tensor(
                                    osb[:], ops[j][:],
                                    ob_sb[:, dmc * 512:(dmc + 1) * 512],
                                    mybir.AluOpType.add)
                                nc.scalar.dma_start(
                                    out_d[r0 + j * P:r0 + (j + 1) * P,
                                          dmc * 512:(dmc + 1) * 512],
                                    osb[:])
                        else:
                            osb = osbp.tile([P, GRP, 512], F32)
                            for j in range(GRP):
                                nc.vector.tensor_tensor(
                                    osb[:, j, :], ops[j][:],
                                    ob_sb[:, dmc * 512:(dmc + 1) * 512],
                                    mybir.AluOpType.add)
                            nc.scalar.dma_start(
                                out_d[r0:r0 + GRP * P,
                                      dmc * 512:(dmc + 1) * 512].rearrange(
                                          "(s p) n -> p s n", p=P),
                                osb[:])
                if t + 1 < NBT:
                    xT_cur = xT_next
    nc.compile()
    return nc


_NC_CACHE = {}


def _get_nc(key):
    if key not in _NC_CACHE:
        cfg = dict(BT=key[0])
        _NC_CACHE[key] = build_nc(cfg)
    return _NC_CACHE[key]


CONFIG = (1024,)


def _softplus(v):
    return np.logaddexp(0.0, v)


def prep_inputs(x, in_w, in_b, conv_w, conv_b, A_log, B, C, Dp, dt,
                out_w, out_b):
    """Host-side preprocessing: per-core input dicts (shared weight arrays)."""
    import ml_dtypes
    BF = ml_dtypes.bfloat16
    x = np.asarray(x, dtype=np.float32)
    in_w = np.asarray(in_w, dtype=np.float32)
    out_w = np.asarray(out_w, dtype=np.float32)

    # host precompute of the per-channel SSM/conv collapse
    c = (np.asarray(conv_w, np.float32)[:, -1]
         + _softplus(np.asarray(dt, np.float32))
         * np.sum(np.asarray(B, np.float32) * np.asarray(C, np.float32), -1)
         + np.asarray(Dp, np.float32))
    b_eff = np.asarray(in_b, np.float32) * c + np.asarray(conv_b, np.float32)

    # [128, DI//128] partition-major layouts for per-partition scale/bias
    c_pb = np.ascontiguousarray(c.reshape(DI // P, P).T)
    b_pb = np.ascontiguousarray(b_eff.reshape(DI // P, P).T)
    ob_rep = np.ascontiguousarray(
        np.broadcast_to(np.asarray(out_b, np.float32), (P, DM)))

    # x^T per core in bf16: [8, DM, BS]
    xt = np.ascontiguousarray(
        x.reshape(N_CORES, BS, DM).transpose(0, 2, 1).astype(BF))
    # in_w packed [p, di, kt*128] so each stationary tile is one
    # contiguous 4KB-per-partition line: in_w[kt*128+p, di*128+m]
    KT = DM // P
    NDI = DI // P
    iw_pack = np.ascontiguousarray(
        in_w.reshape(KT, P, NDI, P).transpose(1, 2, 0, 3)
        .reshape(P, NDI * KT * P).astype(BF))
    ow_arr = np.ascontiguousarray(out_w.astype(BF))

    in_maps = []
    for i in range(N_CORES):
        in_maps.append({
            "xt": xt[i],
            "iw": iw_pack,
            "ow": ow_arr,
            "cpb": c_pb,
            "bpb": b_pb,
            "ob": ob_rep,
        })
    return in_maps


def kernel(x, in_w, in_b, conv_w, conv_b, A_log, B, C, Dp, dt, out_w, out_b):
    in_maps = prep_inputs(x, in_w, in_b, conv_w, conv_b, A_log, B, C, Dp,
                          dt, out_w, out_b)
    nc = _get_nc(CONFIG)
    out = np.empty((B_FULL, DM), dtype=np.float32)
    try:
        res = run_bass_kernel_spmd(nc, in_maps, core_ids=list(range(N_CORES)))
        for i in range(N_CORES):
            out[i * BS:(i + 1) * BS] = res.results[i]["out"]
    except Exception:
        # The accelerator occasionally hits a transient unrecoverable fault
        # that poisons this process's PJRT client; a fresh process recovers.
        # Retry the device execution in a subprocess.
        _run_in_subprocess(in_maps, out)
    return out


def _run_in_subprocess(in_maps, out):
    import pickle
    import subprocess
    import sys
    import tempfile

    with tempfile.TemporaryDirectory() as td:
        in_path = f"{td}/in.pkl"
        out_path = f"{td}/out.npy"
        with open(in_path, "wb") as f:
            pickle.dump({"config": CONFIG, "in_maps": in_maps}, f,
                        protocol=pickle.HIGHEST_PROTOCOL)
        for attempt in range(3):
            r = subprocess.run(
                [sys.executable, __file__, "--worker", in_path, out_path],
                capture_output=True)
            if r.returncode == 0:
                break
            if attempt == 2:
                raise RuntimeError(
                    f"device worker failed 3x: {r.stderr[-2000:]!r}")
        out[:] = np.load(out_path)


def _worker_main(in_path, out_path):
    import pickle
    with open(in_path, "rb") as f:
        job = pickle.load(f)
    nc = _get_nc(tuple(job["config"]))
    res = run_bass_kernel_spmd(nc, job["in_maps"],
                               core_ids=list(range(N_CORES)))
    out = np.empty((B_FULL, DM), dtype=np.float32)
    for i in range(N_CORES):
        out[i * BS:(i + 1) * BS] = res.results[i]["out"]
    np.save(out_path, out)


if __name__ == "__main__":
    import sys as _sys
    if len(_sys.argv) == 4 and _sys.argv[1] == "--worker":
        _worker_main(_sys.argv[2], _sys.argv[3])
